# revision 3
# baseline (speedup 1.0000x reference)
"""Multi-head attention (B=2, S=4096, E=512, H=8) on 8 trn2 NeuronCores — v2.

Sharding: data-parallel over B, sequence-parallel over queries (1024/core).

Per-core pipeline (all engines balanced):
  - projections: q/k/v bf16 matmuls (K proj done once, hoisted), psum->sbuf
    copies on the gpsimd engine.
  - attention per (pair, qtile, ktile): scores^T = k_h q_h^T as a row-group
    pair (partitions 0-63 / 64-127 execute concurrently on the PE);
    mask applied EITHER as an additive Id-matmul into the scores psum
    (act-path ktiles; mask rows hold 0/-360) followed by exp on the scalar
    engine, OR fused into a custom DVE op exp4(u)=((1+u+c1 u^2)^4)*mask
    (dve-path ktiles; mask rows hold 0/1).  12/32 ktiles go to the DVE to
    balance scalar vs vector engine load.
  - PV accumulated in psum with a ones-column for the softmax denominator;
    PV matmuls are emitted with a 2-iteration lag so the tensor engine
    never waits on exp.
  - normalize: denominator broadcast via a K=1 ones matmul (no DRAM
    bounce), reciprocal_approx_fast, multiply into attn (bf16).
  - output projection + bias rank-1 matmul as before.
"""

import math

import ml_dtypes
import numpy as np

B, S, E, H = 2, 4096, 512, 8
HD = E // H  # 64
P = 128
NCORES = 8
QC = (B * S) // NCORES  # 1024
NKT = S // P            # 32
NQT = QC // 512         # 2
NPAIR = H // 2          # 4
SCALE = 1.0 / math.sqrt(E)
EXP_C1 = 0.4922
MASK_NEG = -360.0
DVE_KTS = frozenset(kt for kt in range(NKT) if kt % 2 == 1)
LAG = 3
BF16 = ml_dtypes.bfloat16

_CACHE = {}
LAST_RESULT = None


def _register_exp4_mask():
    import concourse.dve_ops as dve_ops
    from concourse.dve_spec import Spec, Src0, Src1, C0, C1, One, sq, lower
    from concourse.dve_uop import DveOpSpec

    for op in dve_ops.OPS:
        if op.name == "EXP4_MASK_ANT":
            return op

    u = Src0 * C0
    body = sq(sq((C1 * u + One) * u + One)) * Src1

    def ref(in0, in1, s0, s1, imm2):
        uu = in0.astype(np.float32) * s0
        p = (s1 * uu + 1.0) * uu + 1.0
        return ((p * p) ** 2) * in1

    op = dve_ops.DveOp(
        "EXP4_MASK_ANT", Spec(body=body, reference=ref),
        subdim=False, uops_sha={},
    )
    row = max(dve_ops._SUB_OPCODE_FOR_NAME.values()) + 1
    assert row < 0x20
    dve_ops._SUB_OPCODE_FOR_NAME[op.name] = row
    dve_ops.OPS.append(op)
    dve_ops.CUSTOM_DVE_SPECS[op.name] = op.spec
    for ver in ("v3", "v4"):
        spec_c = DveOpSpec(
            name=op.name, opcode=row, uops=lower(op.spec, ver=ver), rd1_en=True,
        )
        op.uops_sha[ver] = spec_c.sha(ver)
    return op


def _build():
    if "nc" in _CACHE:
        return _CACHE["nc"]

    import concourse.tile as tile
    from concourse import bacc, mybir

    exp_op = _register_exp4_mask()

    f32 = mybir.dt.float32
    bf16 = mybir.dt.bfloat16
    Exp = mybir.ActivationFunctionType.Exp

    nc = bacc.Bacc(
        "TRN2", target_bir_lowering=False, debug=False, num_devices=NCORES
    )

    maskT = nc.dram_tensor("maskT", [S, QC], bf16, kind="ExternalInput").ap()
    keysT = nc.dram_tensor("keysT", [E, S], bf16, kind="ExternalInput").ap()
    valsT = nc.dram_tensor("valsT", [E, S], bf16, kind="ExternalInput").ap()
    qryT = nc.dram_tensor("qryT", [E, QC], bf16, kind="ExternalInput").ap()
    wqT = nc.dram_tensor("wqT", [E, E], bf16, kind="ExternalInput").ap()
    wkT = nc.dram_tensor("wkT", [E, E], bf16, kind="ExternalInput").ap()
    wvT = nc.dram_tensor("wvT", [E, E], bf16, kind="ExternalInput").ap()
    woT = nc.dram_tensor("woT", [E, E], bf16, kind="ExternalInput").ap()
    bo = nc.dram_tensor("bo", [E], f32, kind="ExternalInput").ap()
    id64 = nc.dram_tensor("id64", [64, 64], bf16, kind="ExternalInput").ap()
    out = nc.dram_tensor("out", [QC, E], f32, kind="ExternalOutput").ap()

    with tile.TileContext(nc) as tc:
        with tc.tile_pool(name="persist", bufs=1) as persist:
            maskb = persist.tile([P, NKT, QC], bf16)         # 64 KB/part
            v_all = persist.tile([P, NKT, H, HD + 1], bf16)  # 33.3 KB
            kT_all = persist.tile([P, NPAIR, S], bf16)       # 32 KB
            qT_all = persist.tile([P, NPAIR, QC], bf16)      # 8 KB
            attn_all = persist.tile([HD, H, QC], bf16)       # 16 KB
            wo_sb = persist.tile([HD, H, E], bf16)           # 8 KB
            bo_sb = persist.tile([1, E], f32)
            idt = persist.tile([P, 64], bf16)
            ones_t = persist.tile([P, P], f32)
            ones_bf = persist.tile([P, P], bf16)

            nc.sync.dma_start(
                out=wo_sb, in_=woT.rearrange("(h d) o -> d h o", d=HD)
            )
            nc.sync.dma_start(out=bo_sb, in_=bo[None, :])
            nc.sync.dma_start(out=idt[0:64, :], in_=id64)
            nc.sync.dma_start(out=idt[64:128, :], in_=id64)
            nc.vector.memset(ones_t, 1.0)
            nc.vector.memset(ones_bf, 1.0)

            # mask loads spread over gpsimd/scalar DMA queues, in kt order
            for kt in range(NKT):
                nc.gpsimd.dma_start(
                    out=maskb[:, kt, :],
                    in_=maskT[kt * P : (kt + 1) * P, :],
                )

            # ---- v projection ----
            with (
                tc.tile_pool(name="wv", bufs=1) as wvp,
                tc.tile_pool(name="vstage", bufs=4) as vstage,
                tc.tile_pool(name="vps", bufs=2, space="PSUM") as vps,
            ):
                wv_sb = wvp.tile([P, 4, E], bf16)
                nc.sync.dma_start(
                    out=wv_sb, in_=wvT.rearrange("(c p) o -> p c o", p=P)
                )
                vq = (nc.scalar, nc.sync)
                for kt in range(NKT):
                    vs = vstage.tile([P, 4, P], bf16)
                    vq[kt % 2].dma_start(
                        out=vs,
                        in_=valsT[:, kt * P : (kt + 1) * P].rearrange(
                            "(c p) s -> p c s", p=P
                        ),
                    )
                    ps = vps.tile([P, E], f32)
                    for ec in range(4):
                        nc.tensor.matmul(
                            ps,
                            lhsT=vs[:, ec, :],
                            rhs=wv_sb[:, ec, :],
                            start=(ec == 0),
                            stop=(ec == 3),
                        )
                    nc.scalar.copy(
                        out=v_all[:, kt, :, 0:HD],
                        in_=ps.rearrange("p (h d) -> p h d", h=H),
                    )
            nc.vector.memset(v_all[:, :, :, HD : HD + 1], 1.0)

            # ---- k projection (hoisted, single pass) ----
            with (
                tc.tile_pool(name="wk", bufs=1) as wkp,
                tc.tile_pool(name="kstage", bufs=3) as kstage,
                tc.tile_pool(name="kps", bufs=2, space="PSUM") as kps,
            ):
                wk_sb = wkp.tile([P, 4, E], bf16)
                nc.sync.dma_start(
                    out=wk_sb, in_=wkT.rearrange("(c p) o -> p c o", p=P)
                )
                kq = (nc.sync, nc.scalar)
                for kt8 in range(S // 512):
                    ks = kstage.tile([P, 4, 512], bf16)
                    kq[kt8 % 2].dma_start(
                        out=ks,
                        in_=keysT[:, kt8 * 512 : (kt8 + 1) * 512].rearrange(
                            "(c2 p) s -> p c2 s", p=P
                        ),
                    )
                    for c in range(NPAIR):
                        ps = kps.tile([P, 512], f32)
                        for ec in range(4):
                            nc.tensor.matmul(
                                ps,
                                lhsT=wk_sb[:, ec, c * P : (c + 1) * P],
                                rhs=ks[:, ec, :],
                                start=(ec == 0),
                                stop=(ec == 3),
                            )
                        nc.vector.tensor_copy(
                            out=kT_all[:, c, kt8 * 512 : (kt8 + 1) * 512],
                            in_=ps,
                        )

            # ---- q projection ----
            with (
                tc.tile_pool(name="wq", bufs=1) as wqp,
                tc.tile_pool(name="qstage", bufs=2) as qstage,
                tc.tile_pool(name="qps", bufs=2, space="PSUM") as qps,
            ):
                wq_sb = wqp.tile([P, 4, E], bf16)
                nc.sync.dma_start(
                    out=wq_sb, in_=wqT.rearrange("(c p) o -> p c o", p=P)
                )
                for qt in range(NQT):
                    qs = qstage.tile([P, 4, 512], bf16)
                    nc.scalar.dma_start(
                        out=qs,
                        in_=qryT[:, qt * 512 : (qt + 1) * 512].rearrange(
                            "(c p) s -> p c s", p=P
                        ),
                    )
                    for c in range(NPAIR):
                        ps = qps.tile([P, 512], f32)
                        for ec in range(4):
                            nc.tensor.matmul(
                                ps,
                                lhsT=wq_sb[:, ec, c * P : (c + 1) * P],
                                rhs=qs[:, ec, :],
                                start=(ec == 0),
                                stop=(ec == 3),
                            )
                        nc.vector.tensor_copy(
                            out=qT_all[:, c, qt * 512 : (qt + 1) * 512], in_=ps
                        )

            # ---- attention ----
            with (
                tc.tile_pool(name="scps", bufs=2, space="PSUM") as scps,
                tc.tile_pool(name="pvps", bufs=3, space="PSUM") as pvps,
                tc.tile_pool(name="denps", bufs=1, space="PSUM") as denps,
                tc.tile_pool(name="pp", bufs=LAG + 2) as pp,
                tc.tile_pool(name="norm", bufs=2) as norm,
            ):
                for c in range(NPAIR):
                    for qt in range(NQT):
                        qsl = slice(qt * 512, (qt + 1) * 512)
                        pv0 = pvps.tile([HD + 1, 512], f32, tag="pv")
                        pv1 = pvps.tile([HD + 1, 512], f32, tag="pv")
                        p_tiles = {}

                        def emit_pv(kt):
                            p_sb = p_tiles.pop(kt)
                            nc.tensor.matmul(
                                pv0,
                                lhsT=v_all[:, kt, 2 * c, :],
                                rhs=p_sb[:, 0, :],
                                start=(kt == 0),
                                stop=(kt == NKT - 1),
                            )
                            nc.tensor.matmul(
                                pv1,
                                lhsT=v_all[:, kt, 2 * c + 1, :],
                                rhs=p_sb[:, 1, :],
                                start=(kt == 0),
                                stop=(kt == NKT - 1),
                            )

                        for kt in range(NKT):
                            ksl = slice(kt * P, (kt + 1) * P)
                            sc = scps.tile([P, 2, 512], f32)
                            act_path = kt not in DVE_KTS
                            for s_ in range(2):
                                nc.tensor.matmul(
                                    sc[:, s_, :],
                                    lhsT=kT_all[
                                        s_ * HD : (s_ + 1) * HD, c, ksl
                                    ],
                                    rhs=qT_all[s_ * HD : (s_ + 1) * HD, c, qsl],
                                    start=True,
                                    stop=not act_path,
                                )
                            if act_path:
                                for s_ in range(2):
                                    nc.tensor.matmul(
                                        sc[0:64, s_, :],
                                        lhsT=idt[0:64, :],
                                        rhs=maskb[0:64, kt, qsl],
                                        start=False,
                                        stop=False,
                                    )
                                    nc.tensor.matmul(
                                        sc[64:128, s_, :],
                                        lhsT=idt[64:128, :],
                                        rhs=maskb[64:128, kt, qsl],
                                        start=False,
                                        stop=True,
                                    )
                            p_sb = pp.tile([P, 2, 512], bf16)
                            if act_path:
                                nc.scalar.activation(p_sb, sc, Exp, scale=SCALE)
                            else:
                                for s_ in range(2):
                                    nc.vector._custom_dve(
                                        exp_op,
                                        out=p_sb[:, s_, :],
                                        in0=sc[:, s_, :],
                                        in1=maskb[:, kt, qsl],
                                        s0=SCALE / 4.0,
                                        s1=EXP_C1,
                                    )
                            p_tiles[kt] = p_sb
                            if kt >= LAG:
                                emit_pv(kt - LAG)
                        for kt in range(NKT - LAG, NKT):
                            emit_pv(kt)

                        for s_, pv in ((0, pv0), (1, pv1)):
                            h = 2 * c + s_
                            pv_sb = norm.tile([HD + 1, 512], f32, tag="pvsb")
                            nc.vector.tensor_copy(out=pv_sb, in_=pv)
                            den16 = norm.tile([HD + 1, 512], bf16, tag="den16")
                            nc.vector.tensor_copy(
                                out=den16[HD : HD + 1, :],
                                in_=pv_sb[HD : HD + 1, :],
                            )
                            den_b = denps.tile([HD, 512], f32, tag="den")
                            nc.tensor.matmul(
                                den_b,
                                lhsT=ones_bf[64:65, 0:HD],
                                rhs=den16[HD : HD + 1, :],
                                start=True,
                                stop=True,
                            )
                            rep_sb = norm.tile([HD, 512], f32, tag="rep")
                            nc.vector.reciprocal_approx_fast(
                                out=rep_sb, in_=den_b
                            )
                            nc.vector.tensor_tensor(
                                out=attn_all[:, h, qsl],
                                in0=pv_sb[0:HD, :],
                                in1=rep_sb,
                                op=mybir.AluOpType.mult,
                            )

            # ---- output projection + bias ----
            with (
                tc.tile_pool(name="ops", bufs=2, space="PSUM") as ops,
                tc.tile_pool(name="osb", bufs=3) as osb,
            ):
                for q8 in range(QC // P):
                    ps = ops.tile([P, E], f32)
                    for h in range(H):
                        nc.tensor.matmul(
                            ps,
                            lhsT=attn_all[:, h, q8 * P : (q8 + 1) * P],
                            rhs=wo_sb[:, h, :],
                            start=(h == 0),
                            stop=False,
                        )
                    nc.tensor.matmul(
                        ps,
                        lhsT=ones_t[0:1, 0:P],
                        rhs=bo_sb,
                        start=False,
                        stop=True,
                    )
                    ob = osb.tile([P, E], f32)
                    nc.scalar.copy(out=ob, in_=ps)
                    nc.gpsimd.dma_start(
                        out=out[q8 * P : (q8 + 1) * P, :], in_=ob
                    )

    nc.compile()
    _CACHE["nc"] = nc
    return nc


def make_in_maps(values, keys, query, mask, Wv, Wk, Wq, Wo, bo):
    values = np.asarray(values, np.float32)
    keys = np.asarray(keys, np.float32)
    query = np.asarray(query, np.float32)
    mask = np.asarray(mask)
    wqT = np.ascontiguousarray(np.asarray(Wq, np.float32).T.astype(BF16))
    wkT = np.ascontiguousarray(np.asarray(Wk, np.float32).T.astype(BF16))
    wvT = np.ascontiguousarray(np.asarray(Wv, np.float32).T.astype(BF16))
    woT = np.ascontiguousarray(np.asarray(Wo, np.float32).T.astype(BF16))
    bo = np.ascontiguousarray(np.asarray(bo, np.float32))
    id64 = np.eye(64, dtype=BF16)

    # mask rows: dve-path ktiles hold 0/1, act-path ktiles hold -360/0
    dve_row = np.zeros(S, dtype=bool)
    for kt in DVE_KTS:
        dve_row[kt * P : (kt + 1) * P] = True

    in_maps = []
    for core in range(NCORES):
        b, qc = core // (NCORES // B), core % (NCORES // B)
        qsl = slice(qc * QC, (qc + 1) * QC)
        m = mask[b, 0, qsl, :].T.astype(np.float32)  # [S, QC]
        menc = np.where(
            dve_row[:, None], m, (m - 1.0) * (-MASK_NEG)
        ).astype(BF16)
        in_maps.append(
            {
                "maskT": np.ascontiguousarray(menc),
                "keysT": np.ascontiguousarray(keys[b].T.astype(BF16)),
                "valsT": np.ascontiguousarray(values[b].T.astype(BF16)),
                "qryT": np.ascontiguousarray(query[b, qsl].T.astype(BF16)),
                "wqT": wqT,
                "wkT": wkT,
                "wvT": wvT,
                "woT": woT,
                "bo": bo,
                "id64": id64,
            }
        )
    return in_maps


def kernel(values, keys, query, mask, Wv, Wk, Wq, Wo, bo):
    global LAST_RESULT
    from concourse.bass_utils import run_bass_kernel_spmd

    nc = _build()
    in_maps = make_in_maps(values, keys, query, mask, Wv, Wk, Wq, Wo, bo)
    res = run_bass_kernel_spmd(nc, in_maps, core_ids=list(range(NCORES)))
    LAST_RESULT = res

    out = np.empty((B, S, E), np.float32)
    for core in range(NCORES):
        b, qc = core // (NCORES // B), core % (NCORES // B)
        out[b, qc * QC : (qc + 1) * QC] = res.results[core]["out"]
    return out
